# revision 40
# baseline (speedup 1.0000x reference)
"""GNN message-passing kernel for Trainium2 (Bass/Tile), 8-core SPMD.

Model (from the reference):
  h0 = relu(x @ W_in.T + b_in).T            # [500, B] -> vertices 0..500
  for l in 1..7:   agg = segment_sum(w_edge * h[edge_src]) ; h_l = relu(agg)
  out = h[out_verts].T @ W_out.T + b_out    # [B, 10]

Device strategy:
  - Data-parallel over batch: 8 cores x 256 columns each.
  - The sparse aggregation is dense matmuls agg = sum_j A[l,j] @ h_j with
    per-(target-layer, source-layer) blocks A[l,j] built on the host.
  - Precision is allocated where the error actually lands (measured per-block
    on the host reference): the input layer h0 has ~3-6x the norm of later
    layers, and the last layer feeds the output head directly. So:
      * BF blocks {(1,0),(2,0),(7,0),(7,1)} run in bf16 against the
        bf16 master h (exact within bf16),
      * all other blocks run fp8(e4m3) DoubleRow A_hi @ h_hi,
      * selected pairs additionally run A_hi @ h_lo where
        h_lo = fp8(relu(psum) - h_hi) is a second fp8 "residual digit",
        computed in ONE scalar_tensor_tensor op straight from PSUM
        (A carries no scale factor, so psum is in true h units).
  - DMA order == PE consumption order; the whole A stream (~9.8 MB at
    360 GB/s) is the roofline and the tensor engine drafts right behind
    it. Only the (7,6) block depends on layer 6, so it is the last thing
    in both the stream and the PE program; everything else interleaves
    earlier as filler.
  - b_out is folded into the head as a ones-vector matmul so the output
    DMA reads PSUM directly.
"""

import sys

try:
    import concourse  # noqa: F401  (provided by the axon site-path)
except ImportError:
    sys.path.insert(0, "/opt/trn_rl_repo")

import numpy as np
from ml_dtypes import bfloat16, float8_e4m3

# ---- problem geometry (fixed by the problem spec) ----
B = 2048            # total batch
NC = 8              # cores
BL = B // NC        # 256 batch columns per core
IN_DIM = 784
PER = 500           # vertices per layer
PAD = 512           # padded vertices per layer (4*128)
L = 8               # layers (layer 0 = input layer)
OUT_DIM = 10
WARMUP_MM = 33

# (target_layer, source_layer) blocks in bf16 (exact vs the bf16 h master)
BF_BLOCKS = ((1, 0), (2, 0), (7, 0), (7, 1))
# (target, source) pairs that get the fp8 residual pass A_hi @ h_lo
HLO_PAIRS = ((2, 1), (3, 0), (3, 1), (4, 0), (5, 0), (6, 0),
             (7, 2), (7, 3), (7, 4), (7, 5))

BF_SET = set(BF_BLOCKS)
HLO_SET = set(HLO_PAIRS)
FP8_SRCS = {l: [j for j in range(l) if (l, j) not in BF_SET]
            for l in range(1, L)}
# sources whose h_lo tile is ever consumed
LO_SRCS = sorted({j for (_, j) in HLO_PAIRS})
# sources whose h_hi tile is ever consumed
HI_SRCS = sorted({j for l in range(1, L) for j in FP8_SRCS[l]})

_PROG = None
_PROG_KEY = None
_LAST_IN_MAPS = None


def _build_program(used_tiles):
    from concourse import bacc, tile
    import concourse.mybir as mybir

    f32 = mybir.dt.float32
    bf16 = mybir.dt.bfloat16
    fp8 = mybir.dt.float8e4
    AF = mybir.ActivationFunctionType
    ALU = mybir.AluOpType
    DR = mybir.MatmulPerfMode.DoubleRow

    n_used = len(used_tiles)
    # (layer, m) positions that need an h_bf tile (layer 6's head tiles
    # read h6_hi against an fp8 W_out instead -- their feat share is tiny)
    need_bf = set()
    for j in (0, 1, 2, 7):
        for m in range(4):
            need_bf.add((j, m))
    for kt in used_tiles:
        if kt // 4 != 6:
            need_bf.add((kt // 4, kt % 4))
    bf_layers = sorted({j for (j, _) in need_bf})
    n6 = len([kt for kt in used_tiles if kt // 4 == 6])

    nc = bacc.Bacc(None, target_bir_lowering=False)

    xT_d = nc.dram_tensor("xT", [128, 7, BL], bf16, kind="ExternalInput")
    win_d = nc.dram_tensor("W_inT", [128, 4, 7, 128], bf16,
                           kind="ExternalInput")
    bin_d = nc.dram_tensor("b_inP", [128, 4], f32, kind="ExternalInput")
    ab_ds = {}
    for (l, j) in BF_BLOCKS:
        ab_ds[(l, j)] = nc.dram_tensor(
            f"AB_{l}_{j}", [128, 4, PAD], bf16, kind="ExternalInput")
    a8_ds = {}
    for l in range(2, L):
        ns = len(FP8_SRCS[l])
        a8_ds[l] = nc.dram_tensor(
            f"A8_{l}", [128, 2 * ns, 2, PAD], fp8, kind="ExternalInput")
    wout_d = nc.dram_tensor("W_outT", [128, n_used, OUT_DIM], bf16,
                            kind="ExternalInput")
    wout8_d = nc.dram_tensor("W_out8", [128, max(n6, 1), OUT_DIM], fp8,
                             kind="ExternalInput")
    bout_d = nc.dram_tensor("b_outP", [1, OUT_DIM], bf16,
                            kind="ExternalInput")
    out_d = nc.dram_tensor("out", [OUT_DIM, BL], f32, kind="ExternalOutput")

    with tile.TileContext(nc) as tc:
        with (
            tc.tile_pool(name="const", bufs=1) as cpool,
            tc.tile_pool(name="hbuf", bufs=1) as hpool,
            tc.tile_pool(name="ps", bufs=5, space="PSUM") as ppool,
            tc.tile_pool(name="ps7", bufs=2, space="PSUM") as p7pool,
            tc.tile_pool(name="pso", bufs=1, space="PSUM") as opool,
        ):
            dma = nc.sync.dma_start

            # ---- SBUF tiles ----
            xt_s = cpool.tile([128, 7, BL], bf16)
            win_s = cpool.tile([128, 4, 7, 128], bf16, name="win")
            bin_s = cpool.tile([128, 4], f32)
            ab_s = {k: cpool.tile([128, 4, PAD], bf16, name=f"ab{k[0]}{k[1]}")
                    for k in BF_BLOCKS}
            a8_s = {l: cpool.tile([128, 2 * len(FP8_SRCS[l]), 2, PAD], fp8,
                                  name=f"a8_{l}")
                    for l in range(2, L)}
            wout_s = cpool.tile([128, n_used, OUT_DIM], bf16)
            wout8_s = cpool.tile([128, max(n6, 1), OUT_DIM], fp8)
            bout_s = cpool.tile([1, OUT_DIM], bf16)
            ones_s = cpool.tile([1, BL], bf16)
            zeros_s = cpool.tile([128, BL], f32)

            # h tensors live as PAIR tiles [128, 2, BL] (m01 / m23) so a
            # consumer only depends on its own pair's producer op, and each
            # pair is written by ONE wide activation op.
            h_bf = {j: [hpool.tile([128, 2, BL], bf16, name=f"hbf{j}_{p}")
                        for p in range(2)] for j in bf_layers if j != 7}
            h_bf7 = [hpool.tile([128, BL], bf16, name=f"hbf7_{m}")
                     for m in range(4)]
            h_hi = {j: [hpool.tile([128, 2, BL], fp8, name=f"hhi{j}_{p}")
                        for p in range(2)] for j in HI_SRCS}
            h_lo = {j: [hpool.tile([128, 2, BL], fp8, name=f"hlo{j}_{p}")
                        for p in range(2)] for j in LO_SRCS}

            # ---- input-stream DMAs, in PE consumption order (SP queue).
            # Layer-7's bf16 blocks only need h0/h1/h2, so they stream (and
            # compute) interleaved between L3..L6 to keep PE fed while the
            # big late A-layers transfer.
            dma(win_s[:, 0:2, :, :], win_d[:, 0:2, :, :])
            dma(xt_s[:], xT_d[:])
            dma(win_s[:, 2:4, :, :], win_d[:, 2:4, :, :])
            dma(bin_s[:], bin_d[:])
            dma(ab_s[(1, 0)][:, 0:2], ab_ds[(1, 0)][:, 0:2])
            dma(ab_s[(1, 0)][:, 2:4], ab_ds[(1, 0)][:, 2:4])
            dma(ab_s[(2, 0)][:, 0:2], ab_ds[(2, 0)][:, 0:2])
            dma(bout_s[:], bout_d[:])
            dma(ab_s[(2, 0)][:, 2:4], ab_ds[(2, 0)][:, 2:4])
            dma(wout_s[:], wout_d[:])
            dma(wout8_s[:], wout8_d[:])
            dma(a8_s[2][:], a8_ds[2][:])
            dma(a8_s[3][:, 0:4], a8_ds[3][:, 0:4])
            dma(a8_s[3][:, 4:6], a8_ds[3][:, 4:6])
            dma(ab_s[(7, 0)][:], ab_ds[(7, 0)][:])
            dma(a8_s[4][:, 0:4], a8_ds[4][:, 0:4])
            dma(a8_s[4][:, 4:8], a8_ds[4][:, 4:8])
            dma(ab_s[(7, 1)][:], ab_ds[(7, 1)][:])
            dma(a8_s[5][:, 0:6], a8_ds[5][:, 0:6])
            dma(a8_s[5][:, 6:10], a8_ds[5][:, 6:10])
            dma(a8_s[7][:, 0:2], a8_ds[7][:, 0:2])   # j=2
            dma(a8_s[6][:, 0:5], a8_ds[6][:, 0:5])
            dma(a8_s[7][:, 2:4], a8_ds[7][:, 2:4])   # j=3
            dma(a8_s[6][:, 5:10], a8_ds[6][:, 5:10])
            dma(a8_s[7][:, 4:6], a8_ds[7][:, 4:6])   # j=4
            dma(a8_s[6][:, 10:12], a8_ds[6][:, 10:12])
            dma(a8_s[7][:, 6:8], a8_ds[7][:, 6:8])   # j=5
            dma(a8_s[7][:, 8:10], a8_ds[7][:, 8:10])  # j=6

            # ---- PE warmup: ramp the clock model while input DMAs land ----
            wu_w = cpool.tile([128, 2, 128], fp8, name="wu_w")
            wu_x = cpool.tile([128, 2, BL], fp8, name="wu_x")
            nc.vector.memset(wu_w[:], 0.0)
            nc.vector.memset(wu_x[:], 0.0)
            nc.vector.memset(zeros_s[:], 0.0)
            nc.vector.memset(ones_s[:], 1.0)
            wu_ps = ppool.tile([128, 2, BL], f32, tag="ps", name="wu_ps")
            for i in range(WARMUP_MM):
                nc.tensor.matmul(
                    wu_ps[:, 0, :], wu_w[:], wu_x[:],
                    start=(i == 0), stop=(i == WARMUP_MM - 1), perf_mode=DR,
                )


            # ---- head bookkeeping ----
            used_by_layer = {}
            for i, kt in enumerate(used_tiles):
                used_by_layer.setdefault(kt // 4, []).append((i, kt))
            pso = opool.tile([OUT_DIM, BL], f32)
            n_head = 1 + n_used
            head_idx = [0]

            def head_mm(lhsT, rhs):
                nc.tensor.matmul(
                    pso[:], lhsT, rhs,
                    start=(head_idx[0] == 0),
                    stop=(head_idx[0] == n_head - 1),
                )
                head_idx[0] += 1

            def emit_head(j, ms=None):
                for i6, (i, kt) in enumerate(used_by_layer.get(j, [])):
                    if ms is not None and (kt % 4) not in ms:
                        continue
                    m = kt % 4
                    if j == 6:
                        head_mm(wout8_s[:, i6, :],
                                h_hi[6][m // 2][:, m % 2, :])
                    elif j == 7:
                        head_mm(wout_s[:, i, :], h_bf7[m])
                    else:
                        head_mm(wout_s[:, i, :], h_bf[j][m // 2][:, m % 2, :])

            # ---- activation producers: one wide op per PAIR --------------
            def pslice(tiles, m):
                return tiles[m // 2][:, m % 2, :]

            def hi_pair(j, pls, p):
                # pair0 on Act, pair1 on DVE -> both pairs land in parallel
                if p == 0:
                    nc.scalar.activation(h_hi[j][p][:], pls[p][:], AF.Relu)
                else:
                    nc.vector.tensor_scalar_max(h_hi[j][p][:], pls[p][:], 0.0)

            def lo_pair(j, pls, p):
                # h_lo = max(psum, 0) - h_hi   (DVE only: stt reads psum)
                nc.vector.scalar_tensor_tensor(
                    h_lo[j][p][:], pls[p][:], 0.0, h_hi[j][p][:],
                    ALU.max, ALU.subtract)

            def bf_pair(j, pls, p):
                if p == 0:
                    nc.scalar.activation(h_bf[j][p][:], pls[p][:], AF.Relu)
                else:
                    nc.vector.tensor_scalar_max(h_bf[j][p][:], pls[p][:], 0.0)

            # ---- input layer: h0 = relu(W_in.T.T @ xT + b_in) ----
            pins = [ppool.tile([128, 2, BL], f32, tag="ps",
                                name=f"pin{i}") for i in range(2)]
            for pair in range(2):
                for m in (2 * pair, 2 * pair + 1):
                    for kt in range(7):
                        nc.tensor.matmul(
                            pslice(pins, m), win_s[:, m, kt, :],
                            xt_s[:, kt, :],
                            start=(kt == 0 and m % 2 == 0),
                            stop=(kt == 6 and m % 2 == 1))
                for m in (2 * pair, 2 * pair + 1):
                    dst_b = h_bf[0][pair][:, m % 2, :]
                    dst_h = h_hi[0][pair][:, m % 2, :]
                    if m in (0, 2):
                        nc.scalar.activation(dst_b, pslice(pins, m), AF.Relu,
                                             bias=bin_s[:, m:m + 1])
                        nc.scalar.activation(dst_h, pslice(pins, m), AF.Relu,
                                             bias=bin_s[:, m:m + 1])
                    else:
                        nc.vector.scalar_tensor_tensor(
                            dst_b, pslice(pins, m), bin_s[:, m:m + 1],
                            zeros_s[:], ALU.add, ALU.max)
                        nc.vector.scalar_tensor_tensor(
                            dst_h, pslice(pins, m), bin_s[:, m:m + 1],
                            zeros_s[:], ALU.add, ALU.max)
            for p in range(2):
                eng = nc.gpsimd if p == 0 else nc.vector
                eng.tensor_sub(h_lo[0][p][:], h_bf[0][p][:], h_hi[0][p][:])
            head_mm(bout_s[:], ones_s[:])  # fold b_out into the head psum
            emit_head(0)

            # ---- hidden layers ----
            # Layer-7 psums are long-lived: its bf16 blocks run interleaved
            # between L3..L6 (matching the DMA stream order above).
            pl7 = [p7pool.tile([128, 2, BL], f32, tag="ps7",
                               name=f"pl7_{i}") for i in range(2)]
            n_mm7 = 2 * 4 + 5 * 2 + 4 * 2  # bf blocks + hi kps + lo kps
            idx7 = [0]

            def mm7(lhsT, rhs, m, dr):
                nc.tensor.matmul(
                    pslice(pl7, m), lhsT, rhs,
                    start=(idx7[0] == 0 and m % 2 == 0),
                    stop=(idx7[0] == n_mm7 - 1 and m % 2 == 1),
                    perf_mode=(DR if dr else None))

            def l7_bf_block(j):
                a = ab_s[(7, j)]
                for kt in range(4):
                    for m in range(4):
                        mm7(a[:, kt, m * 128:(m + 1) * 128],
                            h_bf[j][kt // 2][:, kt % 2, :], m, False)
                    idx7[0] += 1

            def l7_fp8(j, kind, m_major=False):
                pos = FP8_SRCS[7].index(j)
                src = h_hi[j] if kind == "hi" else h_lo[j]
                a = a8_s[7]
                if m_major:
                    i0 = idx7[0]
                    for m in range(4):
                        idx7[0] = i0
                        for p in range(2):
                            mm7(a[:, 2 * pos + p, :, m * 128:(m + 1) * 128],
                                src[p][:], m, True)
                            idx7[0] += 1
                else:
                    for p in range(2):
                        for m in range(4):
                            mm7(a[:, 2 * pos + p, :, m * 128:(m + 1) * 128],
                                src[p][:], m, True)
                        idx7[0] += 1

            for l in range(1, L - 2):
                ops = []
                for j in range(l):
                    if (l, j) in BF_SET:
                        ops.append(("bf", (j, None)))
                    else:
                        pos = FP8_SRCS[l].index(j)
                        ops.append(("hi", (j, pos)))
                        if (l, j) in HLO_SET:
                            ops.append(("lo", (j, pos)))
                n_mm = sum(4 if k == "bf" else 2 for k, _ in ops)
                pls = [ppool.tile([128, 2, BL], f32, tag="ps",
                                   name=f"pl{l}_{i}") for i in range(2)]
                idx = 0  # per-m accumulation index (same for all m)
                for kind, (j, pos) in ops:
                    if kind == "bf":
                        a = ab_s[(l, j)]
                        for kt in range(4):
                            for m in range(4):
                                nc.tensor.matmul(
                                    pslice(pls, m),
                                    a[:, kt, m * 128:(m + 1) * 128],
                                    h_bf[j][kt // 2][:, kt % 2, :],
                                    start=(idx == 0 and m % 2 == 0),
                                    stop=(idx == n_mm - 1 and m % 2 == 1))
                            idx += 1
                    else:
                        src = h_hi[j] if kind == "hi" else h_lo[j]
                        a = a8_s[l]
                        for p in range(2):
                            for m in range(4):
                                nc.tensor.matmul(
                                    pslice(pls, m),
                                    a[:, 2 * pos + p, :,
                                      m * 128:(m + 1) * 128],
                                    src[p][:],
                                    start=(idx == 0 and m % 2 == 0),
                                    stop=(idx == n_mm - 1 and m % 2 == 1),
                                    perf_mode=DR)
                            idx += 1
                assert idx == n_mm, (l, idx, n_mm)

                for p in range(2):
                    hi_pair(l, pls, p)
                if l in LO_SRCS:
                    for p in range(2):
                        lo_pair(l, pls, p)
                if any((l, m) in need_bf for m in range(4)):
                    for p in range(2):
                        bf_pair(l, pls, p)
                emit_head(l)
                # interleave layer-7 work right after the layer whose
                # DMA precedes it in the stream; everything except the
                # (7,6) block runs before L6
                if l == 3:
                    l7_bf_block(0)
                elif l == 4:
                    l7_bf_block(1)
                elif l == 5:
                    l7_fp8(2, "hi")
                    l7_fp8(2, "lo")

            # ---- layer 6, kp-chunked, with (7,j) passes interleaved so PE
            # consumes the stream exactly in arrival order ----
            n_mm6 = 14  # 12 hi kps + 2 lo kps for source 0
            pls6 = [ppool.tile([128, 2, BL], f32, tag="ps",
                               name=f"pl6_{i}") for i in range(2)]
            idx6 = [0]

            def l6_kps(kps, lo=False):
                src_t = h_lo if lo else h_hi
                for kp in kps:
                    j = kp // 2
                    for m in range(4):
                        nc.tensor.matmul(
                            pslice(pls6, m),
                            a8_s[6][:, kp, :, m * 128:(m + 1) * 128],
                            src_t[j][kp % 2][:],
                            start=(idx6[0] == 0 and m % 2 == 0),
                            stop=(idx6[0] == n_mm6 - 1 and m % 2 == 1),
                            perf_mode=DR)
                    idx6[0] += 1

            l6_kps(range(0, 5))
            l7_fp8(3, "hi")
            l7_fp8(3, "lo")
            l6_kps(range(5, 10))
            l7_fp8(4, "hi")
            l7_fp8(4, "lo")
            l6_kps(range(0, 2), lo=True)   # (6,0) residual pass (early data)
            l6_kps(range(10, 12))          # last chunk's kps stay minimal
            assert idx6[0] == n_mm6
            for p in range(2):
                hi_pair(6, pls6, p)
            l7_fp8(5, "hi")
            l7_fp8(5, "lo")

            # ---- layer 7 tail after L6: (7,6) pair-major; per-m bf acts so
            # each head matmul waits only its own 398ns act ----
            pos6 = FP8_SRCS[7].index(6)
            a7 = a8_s[7]
            for pair in range(2):
                for pp in range(2):
                    for m in (2 * pair, 2 * pair + 1):
                        nc.tensor.matmul(
                            pslice(pl7, m),
                            a7[:, 2 * pos6 + pp, :, m * 128:(m + 1) * 128],
                            h_hi[6][pp][:],
                            start=False,
                            stop=(pp == 1 and m % 2 == 1),
                            perf_mode=DR)
                for m in (2 * pair, 2 * pair + 1):
                    if m in (0, 2):
                        nc.scalar.activation(h_bf7[m][:], pslice(pl7, m),
                                             AF.Relu)
                    else:
                        nc.vector.tensor_scalar_max(h_bf7[m][:],
                                                    pslice(pl7, m), 0.0)
            emit_head(6)
            emit_head(7, ms=(0,))
            emit_head(7, ms=(1,))
            emit_head(7, ms=(2,))
            emit_head(7, ms=(3,))
            idx7[0] = n_mm7  # accounted manually above
            assert idx7[0] == n_mm7

            assert head_idx[0] == n_head
            # ---- output epilogue (bias already folded into the psum) ----
            out_s = cpool.tile([OUT_DIM, BL], f32, name="out_s")
            nc.scalar.activation(out_s[:], pso[:], AF.Identity)
            nc.sync.dma_start(out_d[:], out_s[:])

    nc.compile()
    return nc


def _pack_ptiles(arr2d, n_tiles):
    """[n_tiles*128, F] row-major -> [128, n_tiles, F] partition-major."""
    f = arr2d.shape[1]
    return np.ascontiguousarray(
        arr2d.reshape(n_tiles, 128, f).transpose(1, 0, 2)
    )


def kernel(**inputs):
    x = np.asarray(inputs["x"], np.float32)
    W_in = np.asarray(inputs["W_in"], np.float32)
    b_in = np.asarray(inputs["b_in"], np.float32)
    w_edge = np.asarray(inputs["w_edge"], np.float32)
    W_out = np.asarray(inputs["W_out"], np.float32)
    b_out = np.asarray(inputs["b_out"], np.float32)
    edge_src = np.asarray(inputs["edge_src"]).astype(np.int64)
    edge_dst = np.asarray(inputs["edge_dst_local"]).astype(np.int64)
    offsets = np.asarray(inputs["edge_offsets"]).astype(np.int64)
    out_verts = np.asarray(inputs["out_verts"]).astype(np.int64)

    # ---- host-side packing ----
    shared = {}
    for l in range(1, L):
        s, e = int(offsets[l - 1]), int(offsets[l])
        At = np.zeros((l * PAD, PAD), np.float32)  # [src_padded, tgt], UNSCALED
        rows = (edge_src[s:e] // PER) * PAD + (edge_src[s:e] % PER)
        np.add.at(At, (rows, edge_dst[s:e]), w_edge[s:e])
        for (tl, j) in BF_BLOCKS:
            if tl == l:
                blk = At[j * PAD:(j + 1) * PAD]  # [512, 512]
                shared[f"AB_{l}_{j}"] = np.ascontiguousarray(
                    blk.reshape(4, 128, PAD).transpose(1, 0, 2)
                ).astype(bfloat16)
        srcs = FP8_SRCS[l]
        if srcs:
            blks = np.concatenate([At[j * PAD:(j + 1) * PAD] for j in srcs])
            a8 = blks.astype(float8_e4m3)
            ns = len(srcs)
            shared[f"A8_{l}"] = np.ascontiguousarray(
                a8.reshape(2 * ns, 2, 128, PAD).transpose(2, 0, 1, 3))

    K_IN = 7 * 128
    winT = np.zeros((K_IN, PAD), np.float32)
    winT[:IN_DIM, :PER] = W_in.T
    winT_re = np.ascontiguousarray(
        _pack_ptiles(winT, 7).reshape(128, 7, 4, 128).transpose(0, 2, 1, 3)
    ).astype(bfloat16)

    binP = np.zeros((PAD,), np.float32)
    binP[:PER] = b_in
    binP_re = np.ascontiguousarray(binP.reshape(4, 128).T)

    NT = 4 * L
    woutT = np.zeros((NT * 128, OUT_DIM), np.float32)
    pad_idx = (out_verts // PER) * PAD + (out_verts % PER)
    woutT[pad_idx, :] = W_out.T
    used_tiles = tuple(sorted(set(int(t) for t in pad_idx // 128)))
    woutT_re = np.ascontiguousarray(
        _pack_ptiles(woutT, NT)[:, list(used_tiles), :]
    ).astype(bfloat16)

    boutP = np.ascontiguousarray(b_out.reshape(1, OUT_DIM)).astype(bfloat16)
    l6_tiles = [t for t in used_tiles if t // 4 == 6]
    wout8 = np.ascontiguousarray(
        _pack_ptiles(woutT, NT)[:, l6_tiles, :]
    ).astype(float8_e4m3)
    if not l6_tiles:
        wout8 = np.zeros((128, 1, OUT_DIM), float8_e4m3)

    shared.update({
        "W_inT": winT_re,
        "b_inP": binP_re,
        "W_outT": woutT_re,
        "W_out8": wout8,
        "b_outP": boutP,
    })
    in_maps = []
    for c in range(NC):
        xT = np.zeros((K_IN, BL), np.float32)
        xT[:IN_DIM, :] = x[c * BL:(c + 1) * BL, :].T
        in_maps.append({"xT": _pack_ptiles(xT, 7).astype(bfloat16), **shared})

    from concourse.bass_utils import run_bass_kernel_spmd

    global _LAST_IN_MAPS, _PROG, _PROG_KEY
    _LAST_IN_MAPS = in_maps
    if _PROG is None or _PROG_KEY != used_tiles:
        _PROG = _build_program(used_tiles)
        _PROG_KEY = used_tiles
    res = run_bass_kernel_spmd(_PROG, in_maps, list(range(NC)))
    out = np.concatenate(
        [np.asarray(res.results[c]["out"], np.float32).T for c in range(NC)],
        axis=0,
    )
    return np.ascontiguousarray(out)
